# revision 19
# baseline (speedup 1.0000x reference)
"""Trainium2 Bass kernel for nn_AttentionLayers (B=64, L=1024, H=512, E=2H=1024).

  context[b] = softmax_l( relu(cat(hidden[b], enc[b,l]) @ W_attn + b_attn) @ W_v ) @ enc[b]

Strategy (data-parallel over batch, 8 batches per core on 8 cores):
  - split W_attn into W1 (hidden part, [512,512]) and W2 (encoder part, [1024,512]).
  - per core, precompute hbT[h, b] = (hidden @ W1 + b_attn).T once on TensorE (tiny).
  - per batch:
      zT[h, l]   = sum_k W2[k,h] * encT[k,l]        (TensorE, bf16 in / fp32 PSUM)
      energyT    = relu(zT + hbT[:, b])             (ScalarE, bias per partition, bf16 out)
      att[1, l]  = sum_h Wv[h] * energyT[h, l]      (TensorE)
      softmax    = exp(att - max) / sum             (VectorE max/recip, ScalarE exp+accum,
                                                     both reading att straight from PSUM)
      ctx[1, e]  = sum_l w[l] * enc_nat[l, e]       (TensorE, w transposed to columns via PE)
  - enc is supplied from the host in BOTH layouts (natural [l,e] and transposed [e,l]) as
    bf16, pre-packed partition-major so every DMA is one contiguous run per partition;
    no on-chip transpose of the big tensor is needed and all big matmuls contract along
    the partition dim at full rate.
"""

import sys

for _p in ("/opt/trn_rl_repo",):
    if _p not in sys.path:
        sys.path.insert(0, _p)

import numpy as np
import ml_dtypes

BF16 = ml_dtypes.bfloat16

N_CORES = 8
B, L, H = 64, 1024, 512
E = 2 * H            # 1024
NB = B // N_CORES    # 8 batches per core
KT = E // 128        # 8 k-tiles over encoder feature dim
HT = H // 128        # 4 tiles over hidden dim
LT = L // 128        # 8 l-tiles

_CACHE = {}


def _build_program():
    import concourse.tile as tile
    from concourse import bacc, mybir
    from contextlib import ExitStack

    f32 = mybir.dt.float32
    bf = mybir.dt.bfloat16
    AF = mybir.ActivationFunctionType

    nc = bacc.Bacc("TRN2", target_bir_lowering=False, debug=False, enable_asserts=False)

    # all inputs are packed partition-major on the host: row p holds everything
    # partition p needs, contiguously.
    enc_nat = nc.dram_tensor("enc_nat", [NB * 128, LT * E], bf, kind="ExternalInput").ap()
    enc_tr = nc.dram_tensor("enc_tr", [NB * 128, KT * L], bf, kind="ExternalInput").ap()
    hidT_d = nc.dram_tensor("hidT", [128, HT * NB], bf, kind="ExternalInput").ap()
    w1_d = nc.dram_tensor("w1", [128, HT * H], bf, kind="ExternalInput").ap()
    w2_d = nc.dram_tensor("w2", [128, KT * H], bf, kind="ExternalInput").ap()
    b_d = nc.dram_tensor("bvec", [128, HT], f32, kind="ExternalInput").ap()
    wv_d = nc.dram_tensor("wv", [128, HT], bf, kind="ExternalInput").ap()
    ctx_d = nc.dram_tensor("ctx", [NB, E], f32, kind="ExternalOutput").ap()

    with tile.TileContext(nc) as tc, ExitStack() as ctx:
        consts = ctx.enter_context(tc.tile_pool(name="consts", bufs=1))
        nat_pool = ctx.enter_context(tc.tile_pool(name="nat", bufs=2))
        tr_pool = ctx.enter_context(tc.tile_pool(name="tr", bufs=2))
        en_pool = ctx.enter_context(tc.tile_pool(name="en", bufs=2))
        sm_pool = ctx.enter_context(tc.tile_pool(name="sm", bufs=2))
        out_pool = ctx.enter_context(tc.tile_pool(name="outp", bufs=2))
        zps = ctx.enter_context(tc.tile_pool(name="zps", bufs=3, space="PSUM"))
        attps = ctx.enter_context(tc.tile_pool(name="attps", bufs=1, space="PSUM"))
        ctxps = ctx.enter_context(tc.tile_pool(name="ctxps", bufs=1, space="PSUM"))
        smallps = ctx.enter_context(tc.tile_pool(name="smallps", bufs=1, space="PSUM"))

        # ---- PE warm-up: dummy matmuls with no DMA deps keep the PE busy while
        # the first loads land, so HAM un-throttles before the real work ----
        N_WARMUP = 16
        wup = consts.tile([128, 128], bf)
        nc.vector.memset(wup[:, :], 0.0)
        wup_ps = smallps.tile([128, 128], f32, tag="small")
        for _ in range(N_WARMUP):
            nc.tensor.matmul(wup_ps, wup[:, :], wup[:, :], start=True, stop=True)

        # ---- startup loads: interleave W2 k-blocks with batch-0 enc k-tiles in
        # consumption order so the z matmuls can start as soon as possible ----
        w2_sb = consts.tile([128, KT, H], bf)
        enc_t0 = tr_pool.tile([128, KT, L], bf, tag="enc_t")

        def load_pair(k):
            nc.sync.dma_start(w2_sb[:, k, :], w2_d[:, k * H:(k + 1) * H])
            nc.sync.dma_start(enc_t0[:, k, :], enc_tr[0:128, k * L:(k + 1) * L])

        load_pair(0)
        load_pair(1)
        w1_sb = consts.tile([128, HT, H], bf)
        nc.sync.dma_start(w1_sb, w1_d[:, :])
        hidT_sb = consts.tile([128, HT, NB], bf)
        nc.sync.dma_start(hidT_sb, hidT_d[:, :])
        b_sb = consts.tile([128, HT], f32)
        nc.sync.dma_start(b_sb, b_d[:, :])
        wv_sb = consts.tile([128, HT], bf)
        nc.sync.dma_start(wv_sb, wv_d[:, :])
        for k in range(2, KT):
            load_pair(k)
        ident = consts.tile([1, 1], f32)
        nc.vector.memset(ident[:, :], 1.0)

        # hbT[h, b] = (hidden @ W1 + b_attn).T — emitted lazily (after a couple of
        # batch-0 z groups) so its weight loads don't stall the PE stream.
        hbT_sb = consts.tile([128, HT, NB], f32)

        def emit_hb():
            for ht in range(HT):
                hb_ps = smallps.tile([128, NB], f32, tag="small")
                for k in range(HT):
                    nc.tensor.matmul(
                        hb_ps,
                        w1_sb[:, k, ht * 128:(ht + 1) * 128],
                        hidT_sb[:, k, :],
                        start=(k == 0),
                        stop=(k == HT - 1),
                    )
                nc.scalar.activation(
                    hbT_sb[:, ht, :], hb_ps, AF.Identity,
                    bias=b_sb[:, ht:ht + 1], scale=1.0,
                )

        # ---- per-batch pipeline ----
        for b in range(NB):
            if b == 0:
                enc_t = enc_t0
            else:
                enc_t = tr_pool.tile([128, KT, L], bf, tag="enc_t")
                nc.sync.dma_start(enc_t, enc_tr[b * 128:(b + 1) * 128, :])
            enc_n = nat_pool.tile([128, LT, E], bf)
            nc.sync.dma_start(enc_n, enc_nat[b * 128:(b + 1) * 128, :])

            energyT = en_pool.tile([128, HT, L], bf)
            att_ps = attps.tile([1, L], f32)
            negmax_p = sm_pool.tile([1, 2], f32)
            for lc in range(2):
                ls = lc * 512
                for ht in range(HT):
                    zp = zps.tile([128, 512], f32)
                    for k in range(KT):
                        nc.tensor.matmul(
                            zp,
                            w2_sb[:, k, ht * 128:(ht + 1) * 128],
                            enc_t[:, k, ls:ls + 512],
                            start=(k == 0),
                            stop=(k == KT - 1),
                        )
                    if b == 0 and lc == 0 and ht == 0:
                        emit_hb()
                    nc.scalar.activation(
                        energyT[:, ht, ls:ls + 512], zp, AF.Relu,
                        bias=hbT_sb[:, ht, b:b + 1], scale=1.0,
                    )
                for ht in range(HT):
                    nc.tensor.matmul(
                        att_ps[:, ls:ls + 512],
                        wv_sb[:, ht:ht + 1],
                        energyT[:, ht, ls:ls + 512],
                        start=(ht == 0),
                        stop=(ht == HT - 1),
                    )
                # per-chunk negated max straight from PSUM, overlapping the
                # other chunk's matmuls
                nc.vector.tensor_reduce(
                    negmax_p[:, lc:lc + 1], att_ps[:, ls:ls + 512],
                    axis=mybir.AxisListType.X, op=mybir.AluOpType.max, negate=True,
                )

            negmax = sm_pool.tile([1, 1], f32)
            nc.vector.tensor_reduce(
                negmax, negmax_p, axis=mybir.AxisListType.X, op=mybir.AluOpType.min,
            )
            # exp in two halves (reading att from PSUM) so the w transposes can
            # start on the first half while the second is still on ScalarE
            w_row = sm_pool.tile([1, L], f32)
            sumexp2 = sm_pool.tile([1, 2], f32)
            wT_ps = smallps.tile([128, LT], f32, tag="small")
            for lc in range(2):
                ls = lc * 512
                nc.scalar.activation(
                    w_row[:, ls:ls + 512], att_ps[:, ls:ls + 512], AF.Exp,
                    bias=negmax[:, :], scale=1.0,
                    accum_out=sumexp2[:, lc:lc + 1],
                )
                for j in range(4 * lc, 4 * lc + 4):
                    nc.tensor.transpose(
                        wT_ps[:, j:j + 1], w_row[:, j * 128:(j + 1) * 128], ident[:, :]
                    )
            sumexp = sm_pool.tile([1, 1], f32)
            nc.vector.tensor_reduce(
                sumexp, sumexp2, axis=mybir.AxisListType.X, op=mybir.AluOpType.add,
            )
            rec = sm_pool.tile([1, 1], f32)
            nc.vector.reciprocal(rec, sumexp)
            w_cols = sm_pool.tile([128, LT], bf)
            nc.vector.tensor_copy(w_cols, wT_ps)

            # ctx[1, e] = sum_l w[l] enc[l, e]
            ctx_ps = ctxps.tile([1, E], f32)
            for ec in range(2):
                es = ec * 512
                for lt in range(LT):
                    nc.tensor.matmul(
                        ctx_ps[:, es:es + 512],
                        w_cols[:, lt:lt + 1],
                        enc_n[:, lt, es:es + 512],
                        start=(lt == 0),
                        stop=(lt == LT - 1),
                    )
            ctx_sb = out_pool.tile([1, E], f32)
            nc.vector.tensor_scalar_mul(ctx_sb, ctx_ps, rec[:, :])
            nc.sync.dma_start(ctx_d[b:b + 1, :], ctx_sb)

    nc.compile()
    return nc


def _get_program():
    if "nc" not in _CACHE:
        _CACHE["nc"] = _build_program()
    return _CACHE["nc"]


def _pmajor(a, tiles, p=128):
    """[tiles*p, F] -> [p, tiles*F] partition-major packing."""
    t, rem = divmod(a.shape[0], p)
    assert rem == 0 and t == tiles
    f = a.shape[1]
    return np.ascontiguousarray(
        a.reshape(tiles, p, f).transpose(1, 0, 2).reshape(p, tiles * f)
    )


def _prep_in_maps(hidden, encoder_outputs, W_attn, b_attn, W_v):
    hidden = np.asarray(hidden, dtype=np.float32)
    encoder_outputs = np.asarray(encoder_outputs, dtype=np.float32)
    W_attn = np.asarray(W_attn, dtype=np.float32)
    b_attn = np.asarray(b_attn, dtype=np.float32)
    W_v = np.asarray(W_v, dtype=np.float32)

    enc_bf = encoder_outputs.astype(BF16)
    w2 = _pmajor(np.ascontiguousarray(W_attn[H:]).astype(BF16), KT)
    w1 = _pmajor(np.ascontiguousarray(W_attn[:H]).astype(BF16), HT)
    bvec = np.ascontiguousarray(b_attn.reshape(HT, 128).T)
    wv = np.ascontiguousarray(W_v.astype(BF16).reshape(HT, 128).T)

    in_maps = []
    for c in range(N_CORES):
        sl = slice(c * NB, (c + 1) * NB)
        eb = enc_bf[sl]
        # natural [l, e] rows, partition-major per batch: [NB*128, LT*E]
        nat = np.ascontiguousarray(
            eb.reshape(NB, LT, 128, E).transpose(0, 2, 1, 3)
        ).reshape(NB * 128, LT * E)
        # transposed [e, l] rows, partition-major per batch: [NB*128, KT*L]
        tr = np.ascontiguousarray(
            eb.transpose(0, 2, 1).reshape(NB, KT, 128, L).transpose(0, 2, 1, 3)
        ).reshape(NB * 128, KT * L)
        hidT = _pmajor(np.ascontiguousarray(hidden[sl].T).astype(BF16), HT)
        in_maps.append({
            "enc_nat": nat,
            "enc_tr": tr,
            "hidT": hidT,
            "w1": w1,
            "w2": w2,
            "bvec": bvec,
            "wv": wv,
        })
    return in_maps


def _run(inputs, trace=False, tmpdir=None):
    from concourse.bass_utils import run_bass_kernel_spmd

    nc = _get_program()
    in_maps = _prep_in_maps(**inputs)
    res = run_bass_kernel_spmd(
        nc, in_maps, core_ids=list(range(N_CORES)), trace=trace, tmpdir=tmpdir
    )
    out = np.concatenate(
        [np.asarray(res.results[c]["ctx"]) for c in range(N_CORES)], axis=0
    ).astype(np.float32)
    return out.reshape(B, 1, E), res


def kernel(hidden, encoder_outputs, W_attn, b_attn, W_v):
    out, _ = _run(dict(
        hidden=hidden, encoder_outputs=encoder_outputs,
        W_attn=W_attn, b_attn=b_attn, W_v=W_v,
    ))
    return out
